# revision 17
# baseline (speedup 1.0000x reference)
"""DiffiT transformer block kernel for 8 Trainium2 NeuronCores.

Data-parallel over the B=64 window axis (8 windows per core). Activations
are feature-major ([channel, token]) so every linear contracts over the
SBUF partition axis. Q/K stay feature-major with heads packed at a 96-row
stride (so each head's 72 rows sit at 32-aligned partition bases and the
per-head score matmuls can slice them legally); V is produced token-major
into per-head slots with an appended ones-column, so O^T = V_aug.T @ P^T
yields the softmax denominator as row 72. Per-token scalars (LN mean/rstd,
softmax 1/l) are broadcast across partitions with K=1 ones-matmuls on the
PE. Dense matmuls run bf16; the residual stream stays fp32; small fixup
matmuls use float32r (full-rate fp32 at free-dim >= 256).

All biases and the time-token conditioning (c @ qkvt^T + biases) enter as
rank-1 (K=1) matmul fixups folded into the PSUM accumulations.
"""

import math
from contextlib import ExitStack

import numpy as np
import ml_dtypes

import concourse.bass as bass
import concourse.mybir as mybir
import concourse.tile as tile
from concourse import bacc
from concourse import bass_utils

F32 = mybir.dt.float32
F32R = mybir.dt.float32r
BF16 = mybir.dt.bfloat16
NPBF16 = ml_dtypes.bfloat16
AF = mybir.ActivationFunctionType

P = 128
WS = 16
N = 256            # tokens per window
C = 1152           # hidden
H = 16             # heads
DH = 72            # head dim
HS = 96            # head stride in the QK packing (32-aligned, >= DH)
MLP = 4608
EPS = 1e-6
B = 64
NCORES = 8
NW = B // NCORES   # windows per core
KC = C // P        # 9  k-tiles over the hidden dim
QKM = 2 * H * HS // P   # 24 m-tiles over packed Q+K (96-stride)
KOFF = QKM // 2    # first K-side m-tile
M1T = MLP // P     # 36 fc1 row tiles
SCALE = 1.0 / math.sqrt(DH)


def _r(ap):
    """view a 4-byte fp32 AP as float32r for full-rate PE matmuls"""
    return ap.bitcast(F32R)


def _qk_pieces(h):
    """32-aligned partition pieces covering head h's 72 rows in the
    96-stride packing: [(subtile, base, length), ...]; piece legality:
    base 0 any len, base 64 len<=64, base 32/96 len<=32."""
    start, end = HS * h, HS * h + DH
    out = []
    while start < end:
        sub, base = divmod(start, P)
        ln = min(end - start, P - base)
        if base == 64:
            ln = min(ln, 64)
        elif base in (32, 96):
            ln = min(ln, 32)
        elif base != 0:
            raise AssertionError(base)
        out.append((sub, base, ln))
        start += ln
    return out


def build_program(nw=NW, sim_gelu=False):
    nc = bacc.Bacc("TRN2", target_bir_lowering=False, debug=False,
                   num_devices=NCORES)

    # register the layernorm epsilon as a const AP (activation float biases
    # other than 0.0/1.0 need one), same pattern as Bass.__init__
    eps_t = nc.alloc_sbuf_tensor("const-eps", [P, 1], F32)
    nc.gpsimd.memset(eps_t.ap(), EPS)
    nc.const_aps.aps[(F32, EPS)] = eps_t.ap()
    nc.all_engine_barrier()

    def din(name, shape, dt):
        return nc.dram_tensor(name, shape, dt, kind="ExternalInput").ap()

    xT = din("xT", [KC, P, nw, N], F32)          # x, feature-major
    cT = din("cT", [10, P, nw], BF16)            # c augmented with ones row
    wct = din("wct", [10, P, 4224], BF16)        # qkvt^T reordered + bias row
    wqk = din("wqk", [QKM, P, KC, P], BF16)      # qkv^T QK part, 96-stride
    wv = din("wv", [KC, P, C], BF16)             # qkv^T V part
    expb = din("expb", [H, 2, P, N], BF16)       # exp(rel-pos bias)^T per head
    wps = din("wps", [KC, P, H, P], BF16)        # proj^T, head-slot padded
    w1c = din("w1c", [M1T, P, KC, P], BF16)      # fc1^T pre-chunked
    w2 = din("w2", [M1T, P, C], BF16)            # fc2^T
    f1b = din("f1b", [P, M1T], F32)              # fc1 bias, per-partition
    b2 = din("b2", [1, 2 * C], BF16)             # proj_b ++ fc2_b
    outT = nc.dram_tensor("outT", [KC, P, nw, N], F32,
                          kind="ExternalOutput").ap()

    with tile.TileContext(nc) as tc, ExitStack() as ctx:
        keep = ctx.enter_context(tc.tile_pool(name="keep", bufs=1))
        dram = ctx.enter_context(tc.tile_pool(name="dram", bufs=1,
                                              space="DRAM"))

        ones_b = keep.tile([1, 512], BF16, tag="ones_b")  # bf16 rhs of K=1
        ones_c = keep.tile([P, 1], BF16, tag="ones_c")    # lhsT of column sums
        nc.gpsimd.memset(ones_b[:], 1.0)
        nc.gpsimd.memset(ones_c[:], 1.0)
        bias2 = keep.tile([1, 2 * C], BF16, tag="bias2")
        nc.sync.dma_start(bias2[:], b2[:])
        f1bs = keep.tile([P, M1T], F32, tag="f1bs")
        nc.sync.dma_start(f1bs[:], f1b[:])

        tdram = dram.tile([nw, 4224], BF16)
        xpd = dram.tile([KC, P, nw, N], F32)     # x after attention branch

        # ---- phase 0: conditioning T = c_aug @ W_ct ----------------------
        with tc.tile_pool(name="ph0", bufs=2) as p0, \
             tc.tile_pool(name="ph0p", bufs=2, space="PSUM") as pp0:
            caug = p0.tile([P, 10, nw], BF16, tag="caug")
            nc.sync.dma_start(caug[:], cT.rearrange("k p w -> p k w"))
            tsb = p0.tile([8, 4224], BF16, tag="tsb")
            for i in range(9):
                n0, nl = i * 512, min(512, 4224 - i * 512)
                tps = pp0.tile([8, 512], F32, tag="tps")
                for k in range(10):
                    wt = p0.tile([P, 512], BF16, tag="wctt")
                    nc.sync.dma_start(wt[:, :nl], wct[k, :, n0:n0 + nl])
                    nc.tensor.matmul(tps[:nw, :nl], caug[:, k, :], wt[:, :nl],
                                     start=(k == 0), stop=(k == 9))
                nc.scalar.activation(tsb[:nw, n0:n0 + nl], tps[:nw, :nl],
                                     AF.Copy)
            nc.sync.dma_start(tdram[:, :], tsb[:nw, :])

        # ---- helper: layernorm stats -> PSUM broadcast rows --------------
        def ln_rows(pool, psum, xw, tag):
            """xw: [P, KC, N] fp32. Returns (rstd_bcast, negmean_rstd_bcast)
            PSUM [P, N] tiles."""
            ms0 = psum.tile([1, N], F32, tag=tag + "ms0")
            ms1 = psum.tile([1, N], F32, tag=tag + "ms1")
            for s in range(KC):
                xb = pool.tile([P, N], BF16, tag=tag + "xb")
                nc.gpsimd.tensor_copy(xb[:], xw[:, s, :])
                xsq = pool.tile([P, N], BF16, tag=tag + "xsq")
                nc.vector.tensor_mul(xsq[:], xw[:, s, :], xw[:, s, :])
                nc.tensor.matmul(ms0[:], ones_c[:], xb[:],
                                 start=(s == 0), stop=(s == KC - 1))
                nc.tensor.matmul(ms1[:], ones_c[:], xsq[:],
                                 start=(s == 0), stop=(s == KC - 1))
            mean = pool.tile([1, N], F32, tag=tag + "mean")
            e2 = pool.tile([1, N], F32, tag=tag + "e2")
            nc.vector.tensor_scalar_mul(mean[:], ms0[:], 1.0 / C)
            nc.vector.tensor_scalar_mul(e2[:], ms1[:], 1.0 / C)
            var = pool.tile([1, N], F32, tag=tag + "var")
            nc.vector.tensor_mul(var[:], mean[:], mean[:])
            nc.vector.tensor_sub(var[:], e2[:], var[:])
            sd = pool.tile([1, N], F32, tag=tag + "sd")
            nc.scalar.activation(sd[:], var[:], AF.Sqrt, bias=EPS)
            rstd = pool.tile([1, N], BF16, tag=tag + "rstd")
            with nc.allow_low_precision(reason="bf16 LN scale is intended"):
                nc.vector.reciprocal(rstd[:], sd[:])
            bneg = pool.tile([1, N], BF16, tag=tag + "bneg")
            nc.vector.scalar_tensor_tensor(
                bneg[:], mean[:], -1.0, rstd[:],
                mybir.AluOpType.mult, mybir.AluOpType.mult)
            bcr = psum.tile([P, N], F32, tag=tag + "bcr")
            bcb = psum.tile([P, N], F32, tag=tag + "bcb")
            nc.tensor.matmul(bcr[:], ones_b[:1, :P], rstd[:],
                             start=True, stop=True)
            nc.tensor.matmul(bcb[:], ones_b[:1, :P], bneg[:],
                             start=True, stop=True)
            return bcr, bcb

        # ==== attention super-phase (qkst/vsl alive) ======================
        with tc.tile_pool(name="qkv", bufs=1) as qkv:
            qkst = qkv.tile([P, QKM, nw, N], BF16, tag="qkst")  # Q^T,K^T
            # V slots: col 0 = ones (yields softmax denom as O^T row 0,
            # which keeps all partition bases 32-aligned), cols 1:73 = V
            vsl = qkv.tile([P, nw, 2, H, 73], BF16, tag="vsl")
            nc.vector.memset(vsl[:, :, :, :, 0:1], 1.0)

            # ---- phase 1: per window: LN1 -> h; QK; V --------------------
            with tc.tile_pool(name="ph1", bufs=1) as p1, \
                 tc.tile_pool(name="ph1x", bufs=2) as p1x, \
                 tc.tile_pool(name="ph1p", bufs=1, space="PSUM") as pp1, \
                 tc.tile_pool(name="ph1q", bufs=2, space="PSUM") as pp1q, \
                 tc.tile_pool(name="ph1v", bufs=2, space="PSUM") as pp1v:
                for w in range(nw):
                    xw = p1.tile([P, KC, N], F32, tag="xw")
                    nc.sync.dma_start(
                        xw[:], xT[:, :, w, :].rearrange("s p n -> p s n"))
                    t1w = p1.tile([1, 4224], BF16, tag="t1w")
                    nc.sync.dma_start(t1w[:], tdram[w:w + 1, :])
                    bcr, bcb = ln_rows(p1x, pp1, xw, "ln1")
                    hw = p1x.tile([P, KC, N], BF16, tag="hw")
                    for s in range(KC):
                        nc.vector.tensor_mul(hw[:, s, :], xw[:, s, :],
                                             bcr[:])
                        nc.vector.tensor_add(hw[:, s, :], hw[:, s, :],
                                             bcb[:])
                    for m in range(QKM):
                        wt = p1x.tile([P, KC, P], BF16, tag="wqkt")
                        nc.sync.dma_start(wt[:], wqk[m])
                        qs = pp1q.tile([P, N], F32, tag="qs")
                        for k in range(KC):
                            nc.tensor.matmul(qs[:], wt[:, k, :], hw[:, k, :],
                                             start=(k == 0), stop=False)
                        nc.tensor.matmul(qs[:], t1w[:, P * m:P * (m + 1)],
                                         ones_b[:1, :N], start=False,
                                         stop=True)
                        nc.scalar.activation(qkst[:, m, w, :], qs[:], AF.Copy)
                    for nch in range(4):               # V: 4 chunks x 4 heads
                        n0 = nch * 288
                        wvt = p1x.tile([P, KC, 288], BF16, tag="wvt")
                        nc.sync.dma_start(
                            wvt[:],
                            wv[:, :, n0:n0 + 288].rearrange("s p n -> p s n"))
                        for ms in range(2):
                            vs = pp1v.tile([P, 288], F32, tag="vs")
                            tsl = slice(ms * P, (ms + 1) * P)
                            for k in range(KC):
                                nc.tensor.matmul(vs[:], hw[:, k, tsl],
                                                 wvt[:, k, :],
                                                 start=(k == 0), stop=False)
                            nc.tensor.matmul(
                                vs[:], ones_b[:1, :P],
                                t1w[:, 3072 + n0:3072 + n0 + 288],
                                start=False, stop=True)
                            nc.scalar.activation(
                                vsl[:, w, ms, 4 * nch:4 * nch + 4, 1:73],
                                vs[:].rearrange("p (h d) -> p h d", d=72),
                                AF.Copy)

            # ---- phase 2: attention + proj + residual --------------------
            with tc.tile_pool(name="ph2", bufs=2) as p2, \
                 tc.tile_pool(name="ph2b", bufs=2) as p2b, \
                 tc.tile_pool(name="ph2p", bufs=2, space="PSUM") as pp2, \
                 tc.tile_pool(name="ph2q", bufs=1, space="PSUM") as pp2q:
                for w in range(nw):
                    ost = p2.tile([P, H, N], BF16, tag="ost")
                    # rows 72.. are contracted against zero weight rows in
                    # proj but must not hold stale NaNs
                    nc.gpsimd.memset(ost[64:, :, :], 0.0)
                    for h in range(H):
                        ebt = p2b.tile([P, 2, N], BF16, tag="ebt")
                        nc.sync.dma_start(
                            ebt[:], expb[h].rearrange("s p n -> p s n"))
                        pt = p2b.tile([P, 2, N], BF16, tag="pt")
                        pieces = _qk_pieces(h)
                        po = pp2.tile([P, N], F32, tag="po")
                        for ms in range(2):
                            ssp = pp2.tile([P, N], F32, tag="ssp")
                            msl = slice(ms * P, (ms + 1) * P)
                            for i, (sub, base, ln) in enumerate(pieces):
                                nc.tensor.matmul(
                                    ssp[:],
                                    qkst[base:base + ln, KOFF + sub, w, msl],
                                    qkst[base:base + ln, sub, w, :],
                                    start=(i == 0),
                                    stop=(i == len(pieces) - 1),
                                    tile_position=(base, 0))
                            nc.scalar.activation(pt[:, ms, :], ssp[:], AF.Exp,
                                                 scale=SCALE)
                            nc.vector.tensor_mul(pt[:, ms, :], pt[:, ms, :],
                                                 ebt[:, ms, :])
                        for ms in range(2):
                            nc.tensor.matmul(po[:73, :], vsl[:, w, ms, h, :],
                                             pt[:, ms, :],
                                             start=(ms == 0), stop=(ms == 1))
                        linv = p2b.tile([1, N], BF16, tag="linv")
                        with nc.allow_low_precision(
                                reason="bf16 softmax denom is intended"):
                            nc.vector.reciprocal(linv[:], po[0:1, :])
                        pb = pp2q.tile([P, N], F32, tag="pb")
                        nc.tensor.matmul(pb[:73, :], ones_b[:1, :73],
                                         linv[:], start=True, stop=True)
                        nc.scalar.activation(ost[:73, h, :], po[:73, :],
                                             AF.Copy)
                        nc.vector.tensor_mul(ost[:73, h, :], ost[:73, h, :],
                                             pb[:73, :])
                    # proj + residual (in-place on xw2)
                    xw2 = p2.tile([P, KC, N], F32, tag="xw2")
                    nc.sync.dma_start(
                        xw2[:], xT[:, :, w, :].rearrange("s p n -> p s n"))
                    for pc in range(KC):
                        wpt = p2b.tile([P, H, P], BF16, tag="wpt")
                        nc.sync.dma_start(wpt[:], wps[pc])
                        yps = pp2.tile([P, N], F32, tag="yps")
                        for h in range(H):
                            nc.tensor.matmul(yps[:], wpt[:, h, :],
                                             ost[:, h, :],
                                             start=(h == 0), stop=False)
                        nc.tensor.matmul(
                            yps[:], bias2[:1, P * pc:P * (pc + 1)],
                            ones_b[:1, :N], start=False, stop=True)
                        nc.vector.tensor_add(xw2[:, pc, :], xw2[:, pc, :],
                                             yps[:])
                    nc.sync.dma_start(
                        xpd[:, :, w, :].rearrange("s p n -> p s n"), xw2[:])

        # ---- phase 3a: LN2 -> h' (bf16) for all windows ------------------
        with tc.tile_pool(name="hp", bufs=1) as hppool:
            hpall = hppool.tile([P, KC, nw, N], BF16, tag="hpall")
            with tc.tile_pool(name="ph3a", bufs=2) as p3a, \
                 tc.tile_pool(name="ph3ap", bufs=1, space="PSUM") as pp3a:
                for w in range(nw):
                    xpw = p3a.tile([P, KC, N], F32, tag="xpw")
                    nc.sync.dma_start(
                        xpw[:], xpd[:, :, w, :].rearrange("s p n -> p s n"))
                    bcr, bcb = ln_rows(p3a, pp3a, xpw, "ln2")
                    for s in range(KC):
                        nc.vector.tensor_mul(hpall[:, s, w, :], xpw[:, s, :],
                                             bcr[:])
                        nc.vector.tensor_add(hpall[:, s, w, :],
                                             hpall[:, s, w, :], bcb[:])

            # ---- phase 3b: fc1 -> gelu -> fc2 -> residual ----------------
            with tc.tile_pool(name="ph3b", bufs=1) as p3b, \
                 tc.tile_pool(name="ph3w", bufs=2) as p3w, \
                 tc.tile_pool(name="ph3c", bufs=3) as p3c, \
                 tc.tile_pool(name="ph3bp", bufs=2, space="PSUM") as pp3b, \
                 tc.tile_pool(name="ph3bq", bufs=2, space="PSUM") as pp3q:
                w2sb = p3b.tile([P, M1T, C], BF16, tag="w2sb")
                nc.sync.dma_start(w2sb[:], w2.rearrange("k p n -> p k n"))
                for w in range(nw):
                    h2a = p3b.tile([P, M1T, N], BF16, tag="h2a")
                    for m1 in range(M1T):
                        w1t = p3w.tile([P, KC, P], BF16, tag="w1t")
                        nc.sync.dma_start(w1t[:], w1c[m1])
                        ps1 = pp3b.tile([P, N], F32, tag="ps1")
                        for k in range(KC):
                            nc.tensor.matmul(ps1[:], w1t[:, k, :],
                                             hpall[:, k, w, :],
                                             start=(k == 0),
                                             stop=(k == KC - 1))
                        h2c = h2a[:, m1, :]
                        if not sim_gelu:
                            nc.scalar.activation(h2c[:], ps1[:],
                                                 AF.Gelu_apprx_tanh,
                                                 bias=f1bs[:, m1:m1 + 1])
                        else:
                            # CoreSim has no Gelu LUT: composite tanh gelu
                            u = p3c.tile([P, N], F32, tag="gelu_u")
                            nc.vector.tensor_add(
                                u[:], ps1[:],
                                f1bs[:, m1:m1 + 1].to_broadcast((P, N)))
                            t3 = p3c.tile([P, N], F32, tag="gelu_t3")
                            nc.vector.tensor_mul(t3[:], u[:], u[:])
                            nc.vector.tensor_mul(t3[:], t3[:], u[:])
                            nc.vector.scalar_tensor_tensor(
                                t3[:], t3[:], 0.044715, u[:],
                                mybir.AluOpType.mult, mybir.AluOpType.add)
                            nc.scalar.activation(t3[:], t3[:], AF.Tanh,
                                                 scale=0.7978845608028654)
                            nc.vector.scalar_tensor_tensor(
                                t3[:], t3[:], 1.0, u[:],
                                mybir.AluOpType.add, mybir.AluOpType.mult)
                            nc.vector.tensor_scalar_mul(h2c[:], t3[:], 0.5)
                    for pm in range(KC):
                        ps2 = pp3q.tile([P, N], F32, tag="ps2")
                        for m1 in range(M1T):
                            nc.tensor.matmul(
                                ps2[:], w2sb[:, m1, P * pm:P * (pm + 1)],
                                h2a[:, m1, :], start=(m1 == 0), stop=False)
                        nc.tensor.matmul(
                            ps2[:],
                            bias2[:1, C + P * pm:C + P * (pm + 1)],
                            ones_b[:1, :N], start=False, stop=True)
                        xps = p3c.tile([P, N], F32, tag="xps")
                        nc.sync.dma_start(xps[:], xpd[pm, :, w, :])
                        ot = p3c.tile([P, N], F32, tag="ot")
                        nc.vector.tensor_add(ot[:], xps[:], ps2[:])
                        nc.sync.dma_start(outT[pm, :, w, :], ot[:])

    nc.compile()
    return nc


# ---------------------------------------------------------------------------
# host side
# ---------------------------------------------------------------------------

def _qk_colmap():
    m = np.full(2 * H * HS, -1, np.int64)
    for h in range(H):
        m[HS * h:HS * h + DH] = np.arange(72 * h, 72 * h + 72)
        m[H * HS + HS * h:H * HS + HS * h + DH] = \
            np.arange(C + 72 * h, C + 72 * h + 72)
    return m


def _prep_core_inputs(x_c, c_c, wdict):
    """x_c: [nw, N, C], c_c: [nw, C] -> per-core input map"""
    nw = x_c.shape[0]
    xT = np.ascontiguousarray(
        x_c.transpose(2, 0, 1).reshape(KC, P, nw, N)).astype(np.float32)
    caug = np.zeros((nw, 1280), np.float32)
    caug[:, :C] = c_c
    caug[:, C] = 1.0
    cT = np.ascontiguousarray(caug.T.reshape(10, P, nw)).astype(NPBF16)
    return {"xT": xT, "cT": cT, **wdict}


def _prep_weights(qkv_w, qkv_b, qkvt_w, qkvt_b, rpb_table, rel_idx,
                  proj_w, proj_b, fc1_w, fc1_b, fc2_w, fc2_b):
    qkmap = _qk_colmap()
    amap = np.concatenate([qkmap, np.arange(2 * C, 3 * C)])  # 4224 cols
    valid = amap >= 0

    wct = np.zeros((1280, 4224), np.float32)
    wct[:C, valid] = qkvt_w[amap[valid], :].T
    wct[C, valid] = (qkv_b + qkvt_b)[amap[valid]]
    wct = wct.reshape(10, P, 4224).astype(NPBF16)

    nqk = 2 * H * HS
    wqkT = np.zeros((C, nqk), np.float32)
    wqkT[:, valid[:nqk]] = qkv_w[qkmap[valid[:nqk]], :].T
    wqk = np.ascontiguousarray(
        wqkT.reshape(KC, P, QKM, P).transpose(2, 1, 0, 3)).astype(NPBF16)

    wv = np.ascontiguousarray(
        qkv_w[2 * C:, :].T.reshape(KC, P, C)).astype(NPBF16)

    bias = rpb_table[rel_idx]                      # [N(n), N(m), H]
    expb = np.ascontiguousarray(
        np.exp(bias).transpose(2, 1, 0).reshape(H, 2, P, N)).astype(NPBF16)

    wp_sl = np.zeros((P, H, C), np.float32)        # [slot-row d, head, p]
    for h in range(H):
        wp_sl[1:73, h, :] = proj_w[:, 72 * h:72 * h + 72].T
    wps = np.ascontiguousarray(
        wp_sl.reshape(P, H, KC, P).transpose(2, 0, 1, 3)).astype(NPBF16)

    w1c = np.ascontiguousarray(
        fc1_w.T.reshape(KC, P, M1T, P).transpose(2, 1, 0, 3)).astype(NPBF16)
    w2 = np.ascontiguousarray(
        fc2_w.T.reshape(M1T, P, C)).astype(NPBF16)
    f1b = np.ascontiguousarray(fc1_b.reshape(M1T, P).T).astype(np.float32)
    b2 = np.concatenate([proj_b, fc2_b]).reshape(1, 2 * C).astype(NPBF16)

    return {"wct": wct, "wqk": wqk, "wv": wv, "expb": expb, "wps": wps,
            "w1c": w1c, "w2": w2, "f1b": f1b, "b2": b2}


_PROGRAM = None


def kernel(x, c, qkv_w, qkv_b, qkvt_w, qkvt_b, rpb_table, proj_w, proj_b,
           fc1_w, fc1_b, fc2_w, fc2_b, rel_idx, _trace=False):
    global _PROGRAM
    x = np.asarray(x, np.float32)
    c = np.asarray(c, np.float32)
    wdict = _prep_weights(
        np.asarray(qkv_w, np.float32), np.asarray(qkv_b, np.float32),
        np.asarray(qkvt_w, np.float32), np.asarray(qkvt_b, np.float32),
        np.asarray(rpb_table, np.float32), np.asarray(rel_idx),
        np.asarray(proj_w, np.float32), np.asarray(proj_b, np.float32),
        np.asarray(fc1_w, np.float32), np.asarray(fc1_b, np.float32),
        np.asarray(fc2_w, np.float32), np.asarray(fc2_b, np.float32))

    if _PROGRAM is None:
        _PROGRAM = build_program(NW)
    nc = _PROGRAM

    in_maps = []
    for core in range(NCORES):
        sl = slice(core * NW, (core + 1) * NW)
        in_maps.append(_prep_core_inputs(x[sl], c[sl], wdict))

    res = bass_utils.run_bass_kernel_spmd(
        nc, in_maps, core_ids=list(range(NCORES)), trace=_trace)

    out = np.empty((B, N, C), np.float32)
    for core in range(NCORES):
        oT = res.results[core]["outT"]            # [KC, P, NW, N]
        out[core * NW:(core + 1) * NW] = \
            oT.reshape(C, NW, N).transpose(1, 2, 0)
    if _trace:
        return out, res
    return out


# revision 22
# speedup vs baseline: 1.0840x; 1.0840x over previous
"""DiffiT transformer block kernel for 8 Trainium2 NeuronCores.

Data-parallel over the B=64 window axis (8 windows per core). Activations
are feature-major ([channel, token]) so every linear contracts over the
SBUF partition axis. Q/K stay feature-major with heads packed at a 96-row
stride (so each head's 72 rows sit at 32-aligned partition bases and the
per-head score matmuls can slice them legally); V is produced token-major
into per-head slots with an appended ones-column, so O^T = V_aug.T @ P^T
yields the softmax denominator as row 72. Per-token scalars (LN mean/rstd,
softmax 1/l) are broadcast across partitions with K=1 ones-matmuls on the
PE. Dense matmuls run bf16; the residual stream stays fp32; small fixup
matmuls use float32r (full-rate fp32 at free-dim >= 256).

All biases and the time-token conditioning (c @ qkvt^T + biases) enter as
rank-1 (K=1) matmul fixups folded into the PSUM accumulations.
"""

import math
from contextlib import ExitStack

import numpy as np
import ml_dtypes

import concourse.bass as bass
import concourse.mybir as mybir
import concourse.tile as tile
from concourse import bacc
from concourse import bass_utils

F32 = mybir.dt.float32
F32R = mybir.dt.float32r
BF16 = mybir.dt.bfloat16
NPBF16 = ml_dtypes.bfloat16
AF = mybir.ActivationFunctionType

P = 128
WS = 16
N = 256            # tokens per window
C = 1152           # hidden
H = 16             # heads
DH = 72            # head dim
HS = 96            # head stride in the QK packing (32-aligned, >= DH)
MLP = 4608
EPS = 1e-6
B = 64
NCORES = 8
NW = B // NCORES   # windows per core
KC = C // P        # 9  k-tiles over the hidden dim
QKM = 2 * H * HS // P   # 24 m-tiles over packed Q+K (96-stride)
KOFF = QKM // 2    # first K-side m-tile
M1T = MLP // P     # 36 fc1 row tiles
SCALE = 1.0 / math.sqrt(DH)


def _r(ap):
    """view a 4-byte fp32 AP as float32r for full-rate PE matmuls"""
    return ap.bitcast(F32R)


def _qk_pieces(h):
    """32-aligned partition pieces covering head h's 72 rows in the
    96-stride packing: [(subtile, base, length), ...]; piece legality:
    base 0 any len, base 64 len<=64, base 32/96 len<=32."""
    start, end = HS * h, HS * h + DH
    out = []
    while start < end:
        sub, base = divmod(start, P)
        ln = min(end - start, P - base)
        if base == 64:
            ln = min(ln, 64)
        elif base in (32, 96):
            ln = min(ln, 32)
        elif base != 0:
            raise AssertionError(base)
        out.append((sub, base, ln))
        start += ln
    return out


def build_program(nw=NW, sim_gelu=False):
    nc = bacc.Bacc("TRN2", target_bir_lowering=False, debug=False,
                   num_devices=NCORES)

    # register the layernorm epsilon as a const AP (activation float biases
    # other than 0.0/1.0 need one), same pattern as Bass.__init__
    eps_t = nc.alloc_sbuf_tensor("const-eps", [P, 1], F32)
    nc.gpsimd.memset(eps_t.ap(), EPS)
    nc.const_aps.aps[(F32, EPS)] = eps_t.ap()
    nc.all_engine_barrier()

    def din(name, shape, dt):
        return nc.dram_tensor(name, shape, dt, kind="ExternalInput").ap()

    xT = din("xT", [nw, P, KC, N], F32)          # x, feature-major
    cT = din("cT", [10, P, nw], BF16)            # c augmented with ones row
    wct = din("wct", [10, P, 4224], BF16)        # qkvt^T reordered + bias row
    wqk = din("wqk", [QKM, P, KC, P], BF16)      # qkv^T QK part, 96-stride
    wv = din("wv", [4, P, KC, 288], BF16)        # qkv^T V part, chunk-major
    expb = din("expb", [H, P, 2, N], BF16)       # exp(rel-pos bias)^T per head
    wps = din("wps", [KC, P, H, P], BF16)        # proj^T, head-slot padded
    w1c = din("w1c", [M1T, P, KC, P], BF16)      # fc1^T pre-chunked
    w2 = din("w2", [M1T, P, C], BF16)            # fc2^T
    f1b = din("f1b", [P, M1T], F32)              # fc1 bias, per-partition
    b2 = din("b2", [1, 2 * C], BF16)             # proj_b ++ fc2_b
    outT = nc.dram_tensor("outT", [nw, P, KC, N], F32,
                          kind="ExternalOutput").ap()

    NPAIR = nw // 2
    W2N = 2 * N        # tokens per window pair

    with tile.TileContext(nc) as tc, ExitStack() as ctx:
        keep = ctx.enter_context(tc.tile_pool(name="keep", bufs=1))
        dram = ctx.enter_context(tc.tile_pool(name="dram", bufs=1,
                                              space="DRAM"))

        ones_b = keep.tile([1, W2N], BF16, tag="ones_b")  # bf16 rhs of K=1
        ones_c = keep.tile([P, 1], BF16, tag="ones_c")    # lhsT of column sums
        nc.gpsimd.memset(ones_b[:], 1.0)
        nc.gpsimd.memset(ones_c[:], 1.0)
        bias2 = keep.tile([1, 2 * C], BF16, tag="bias2")
        nc.sync.dma_start(bias2[:], b2[:])
        f1bs = keep.tile([P, M1T], F32, tag="f1bs")
        nc.sync.dma_start(f1bs[:], f1b[:])

        tdram = dram.tile([nw, 4224], BF16)
        xpd = dram.tile([nw, P, KC, N], F32)     # x after attention branch

        # ---- phase 0: conditioning T = c_aug @ W_ct ----------------------
        with tc.tile_pool(name="ph0", bufs=2) as p0, \
             tc.tile_pool(name="ph0p", bufs=2, space="PSUM") as pp0:
            caug = p0.tile([P, 10, nw], BF16, tag="caug")
            nc.sync.dma_start(caug[:], cT.rearrange("k p w -> p k w"))
            tsb = p0.tile([8, 4224], BF16, tag="tsb")
            for i in range(9):
                n0, nl = i * 512, min(512, 4224 - i * 512)
                tps = pp0.tile([8, 512], F32, tag="tps")
                for k in range(10):
                    wt = p0.tile([P, 512], BF16, tag="wctt")
                    nc.sync.dma_start(wt[:, :nl], wct[k, :, n0:n0 + nl])
                    nc.tensor.matmul(tps[:nw, :nl], caug[:, k, :], wt[:, :nl],
                                     start=(k == 0), stop=(k == 9))
                nc.scalar.activation(tsb[:nw, n0:n0 + nl], tps[:nw, :nl],
                                     AF.Copy)
            nc.sync.dma_start(tdram[:, :], tsb[:nw, :])

        # ---- layernorm for a window pair -> PSUM broadcast [P, W2N] ------
        # acc-tile layout: [:, :N]+[:, N:] hold the two windows; returns one
        # [P, W2N] psum tile pair (rstd bcast, -mean*rstd bcast)
        def ln_pair(pool, rowp, accp, xw, tag):
            rstd = pool.tile([1, W2N], BF16, tag=tag + "rstd")
            bneg = pool.tile([1, W2N], BF16, tag=tag + "bneg")
            for wh in range(2):
                nsl = slice(wh * N, (wh + 1) * N)
                ms0 = rowp.tile([1, 512], F32, tag=tag + "row")
                ms1 = rowp.tile([1, 512], F32, tag=tag + "row")
                for s in range(KC):
                    xb = pool.tile([P, N], BF16, tag=tag + "xb")
                    nc.gpsimd.tensor_copy(xb[:], xw[:, s, nsl])
                    xsq = pool.tile([P, N], BF16, tag=tag + "xsq")
                    nc.vector.tensor_mul(xsq[:], xw[:, s, nsl], xw[:, s, nsl])
                    nc.tensor.matmul(ms0[:, :N], ones_c[:], xb[:],
                                     start=(s == 0), stop=(s == KC - 1))
                    nc.tensor.matmul(ms1[:, :N], ones_c[:], xsq[:],
                                     start=(s == 0), stop=(s == KC - 1))
                mean = pool.tile([1, N], F32, tag=tag + "mean")
                e2 = pool.tile([1, N], F32, tag=tag + "e2")
                nc.vector.tensor_scalar_mul(mean[:], ms0[:, :N], 1.0 / C)
                nc.vector.tensor_scalar_mul(e2[:], ms1[:, :N], 1.0 / C)
                var = pool.tile([1, N], F32, tag=tag + "var")
                nc.vector.tensor_mul(var[:], mean[:], mean[:])
                nc.vector.tensor_sub(var[:], e2[:], var[:])
                sd = pool.tile([1, N], F32, tag=tag + "sd")
                nc.scalar.activation(sd[:], var[:], AF.Sqrt, bias=EPS)
                with nc.allow_low_precision(reason="bf16 LN scale intended"):
                    nc.vector.reciprocal(rstd[:, nsl], sd[:])
                nc.vector.scalar_tensor_tensor(
                    bneg[:, nsl], mean[:], -1.0, rstd[:, nsl],
                    mybir.AluOpType.mult, mybir.AluOpType.mult)
            bc = accp.tile([P, W2N], F32, tag="acc")
            nc.tensor.matmul(bc[:, :N], ones_b[:1, :P], rstd[:1, :N],
                             start=True, stop=True)
            nc.tensor.matmul(bc[:, N:], ones_b[:1, :P], rstd[:1, N:],
                             start=True, stop=True)
            bb = accp.tile([P, W2N], F32, tag="acc")
            nc.tensor.matmul(bb[:, :N], ones_b[:1, :P], bneg[:1, :N],
                             start=True, stop=True)
            nc.tensor.matmul(bb[:, N:], ones_b[:1, :P], bneg[:1, N:],
                             start=True, stop=True)
            return bc, bb

        # ==== attention superphase: per pair LN1 -> QKV -> attn -> proj ===
        with tc.tile_pool(name="sp", bufs=2) as sp, \
             tc.tile_pool(name="sp1", bufs=1) as sp1, \
             tc.tile_pool(name="spw", bufs=2) as spw, \
             tc.tile_pool(name="sps", bufs=3) as sps, \
             tc.tile_pool(name="rowp", bufs=2, space="PSUM") as rowp, \
             tc.tile_pool(name="accp", bufs=6, space="PSUM") as accp:
            for pr in range(NPAIR):
                w0 = 2 * pr
                xw = sp.tile([P, KC, W2N], F32, tag="xw")
                for wh in range(2):
                    nc.sync.dma_start(
                        xw[:, :, wh * N:(wh + 1) * N], xT[w0 + wh])
                t1w = sp1.tile([1, 2, 4224], BF16, tag="t1w")
                nc.sync.dma_start(t1w[:], tdram[w0:w0 + 2, :]
                                  .unsqueeze(0))
                bc, bb = ln_pair(spw, rowp, accp, xw, "ln1")
                hw = sp.tile([P, KC, W2N], BF16, tag="hw")
                for s in range(KC):
                    nc.vector.tensor_mul(hw[:, s, :], xw[:, s, :], bc[:])
                    nc.vector.tensor_add(hw[:, s, :], hw[:, s, :], bb[:])
                # QK (96-stride packed), N = both windows
                qkst = sp.tile([P, QKM, W2N], BF16, tag="qkst")
                for m in range(QKM):
                    wt = spw.tile([P, KC, P], BF16, tag="wqkt")
                    nc.sync.dma_start(wt[:], wqk[m])
                    qs = accp.tile([P, W2N], F32, tag="acc")
                    for k in range(KC):
                        nc.tensor.matmul(qs[:], wt[:, k, :], hw[:, k, :],
                                         start=(k == 0), stop=False)
                    msl = slice(P * m, P * (m + 1))
                    nc.tensor.matmul(qs[:, :N], t1w[:1, 0, msl],
                                     ones_b[:1, :N], start=False, stop=False)
                    nc.tensor.matmul(qs[:, N:], t1w[:1, 1, msl],
                                     ones_b[:1, :N], start=False, stop=True)
                    nc.scalar.activation(qkst[:, m, :], qs[:], AF.Copy)
                # V token-major into per-head slots (ones in col 0)
                vsl = sp1.tile([P, 2, 2, H, 73], BF16, tag="vsl")
                nc.vector.memset(vsl[:, :, :, :, 0:1], 1.0)
                for nch in range(4):
                    wvt = spw.tile([P, KC, 288], BF16, tag="wvt")
                    nc.sync.dma_start(wvt[:], wv[nch])
                    for tch in range(4):       # token chunks of the pair
                        wh, ms = divmod(tch, 2)
                        vs = accp.tile([P, W2N], F32, tag="acc")
                        tsl = slice(tch * P, (tch + 1) * P)
                        for k in range(KC):
                            nc.tensor.matmul(vs[:, :288], hw[:, k, tsl],
                                             wvt[:, k, :],
                                             start=(k == 0), stop=False)
                        nc.tensor.matmul(
                            vs[:, :288], ones_b[:1, :P],
                            t1w[:1, wh, 3072 + 288 * nch:3072 + 288 * (nch + 1)],
                            start=False, stop=True)
                        nc.scalar.activation(
                            vsl[:, wh, ms, 4 * nch:4 * nch + 4, 1:73],
                            vs[:, :288].rearrange("p (h d) -> p h d", d=72),
                            AF.Copy)
                # attention per (window, head)
                ost = sp1.tile([P, H, W2N], BF16, tag="ost")
                nc.gpsimd.memset(ost[64:, :, :], 0.0)
                for wh in range(2):
                    nsl = slice(wh * N, (wh + 1) * N)
                    for h in range(H):
                        ebt = sps.tile([P, 2, N], BF16, tag="ebt")
                        nc.sync.dma_start(ebt[:], expb[h])
                        pt = sps.tile([P, 2, N], BF16, tag="pt")
                        pieces = _qk_pieces(h)
                        po = accp.tile([P, W2N], F32, tag="acc")
                        for ms in range(2):
                            ssp = accp.tile([P, W2N], F32, tag="acc")
                            msl = slice(wh * N + ms * P, wh * N + (ms + 1) * P)
                            for i, (sub, base, ln) in enumerate(pieces):
                                nc.tensor.matmul(
                                    ssp[:, :N],
                                    qkst[base:base + ln, KOFF + sub, msl],
                                    qkst[base:base + ln, sub, nsl],
                                    start=(i == 0),
                                    stop=(i == len(pieces) - 1),
                                    tile_position=(base, 0))
                            nc.scalar.activation(pt[:, ms, :], ssp[:, :N],
                                                 AF.Exp, scale=SCALE)
                            nc.vector.tensor_mul(pt[:, ms, :], pt[:, ms, :],
                                                 ebt[:, ms, :])
                        for ms in range(2):
                            nc.tensor.matmul(po[:73, :N],
                                             vsl[:, wh, ms, h, :],
                                             pt[:, ms, :],
                                             start=(ms == 0), stop=(ms == 1))
                        linv = sps.tile([1, N], BF16, tag="linv")
                        with nc.allow_low_precision(
                                reason="bf16 softmax denom intended"):
                            nc.vector.reciprocal(linv[:], po[0:1, :N])
                        pb = accp.tile([P, W2N], F32, tag="acc")
                        nc.tensor.matmul(pb[:73, :N], ones_b[:1, :73],
                                         linv[:], start=True, stop=True)
                        nc.scalar.activation(ost[:73, h, nsl], po[:73, :N],
                                             AF.Copy)
                        nc.vector.tensor_mul(ost[:73, h, nsl],
                                             ost[:73, h, nsl], pb[:73, :N])
                # proj + residual (in place on xw), both windows at once
                for pc in range(KC):
                    wpt = spw.tile([P, H, P], BF16, tag="wpt")
                    nc.sync.dma_start(wpt[:], wps[pc])
                    yps = accp.tile([P, W2N], F32, tag="acc")
                    for h in range(H):
                        nc.tensor.matmul(yps[:], wpt[:, h, :], ost[:, h, :],
                                         start=(h == 0), stop=False)
                    nc.tensor.matmul(yps[:], bias2[:1, P * pc:P * (pc + 1)],
                                     ones_b[:1, :W2N], start=False, stop=True)
                    nc.vector.tensor_add(xw[:, pc, :], xw[:, pc, :], yps[:])
                for wh in range(2):
                    nc.sync.dma_start(
                        xpd[w0 + wh],
                        xw[:, :, wh * N:(wh + 1) * N])

        # ---- phase 3a: LN2 -> h' (bf16) for all windows ------------------
        with tc.tile_pool(name="hp", bufs=1) as hppool:
            hpall = hppool.tile([P, KC, nw, N], BF16, tag="hpall")
            with tc.tile_pool(name="ph3a", bufs=2) as p3a, \
                 tc.tile_pool(name="rp3", bufs=2, space="PSUM") as rp3, \
                 tc.tile_pool(name="ap3", bufs=2, space="PSUM") as ap3:
                for pr in range(NPAIR):
                    w0 = 2 * pr
                    xpw = p3a.tile([P, KC, W2N], F32, tag="xpw")
                    for wh in range(2):
                        nc.sync.dma_start(
                            xpw[:, :, wh * N:(wh + 1) * N], xpd[w0 + wh])
                    bc, bb = ln_pair(p3a, rp3, ap3, xpw, "ln2")
                    xpv = xpw[:].rearrange("p s (u n) -> p s u n", n=N)
                    for s in range(KC):
                        nc.vector.tensor_mul(
                            hpall[:, s, w0:w0 + 2, :], xpv[:, s, :, :],
                            bc[:].rearrange("p (u n) -> p u n", n=N))
                        nc.vector.tensor_add(
                            hpall[:, s, w0:w0 + 2, :],
                            hpall[:, s, w0:w0 + 2, :],
                            bb[:].rearrange("p (u n) -> p u n", n=N))

            # ---- phase 3b: fc1 -> gelu -> fc2 -> residual, per pair ------
            with tc.tile_pool(name="ph3b", bufs=1) as p3b, \
                 tc.tile_pool(name="ph3w", bufs=2) as p3w, \
                 tc.tile_pool(name="ph3c", bufs=3) as p3c, \
                 tc.tile_pool(name="ph3bp", bufs=2, space="PSUM") as pp3b, \
                 tc.tile_pool(name="ph3bq", bufs=2, space="PSUM") as pp3q:
                w2sb = p3b.tile([P, M1T, C], BF16, tag="w2sb")
                nc.sync.dma_start(w2sb[:], w2.rearrange("k p n -> p k n"))
                for pr in range(NPAIR):
                    w0 = 2 * pr
                    h2a = p3b.tile([P, M1T, W2N], BF16, tag="h2a")
                    hin = hpall[:, :, w0:w0 + 2, :].rearrange(
                        "p s u n -> p s (u n)")
                    for m1 in range(M1T):
                        w1t = p3w.tile([P, KC, P], BF16, tag="w1t")
                        nc.sync.dma_start(w1t[:], w1c[m1])
                        ps1 = pp3b.tile([P, W2N], F32, tag="ps1")
                        for k in range(KC):
                            nc.tensor.matmul(ps1[:], w1t[:, k, :],
                                             hin[:, k, :],
                                             start=(k == 0),
                                             stop=(k == KC - 1))
                        h2c = h2a[:, m1, :]
                        if not sim_gelu:
                            nc.scalar.activation(h2c, ps1[:],
                                                 AF.Gelu_apprx_tanh,
                                                 bias=f1bs[:, m1:m1 + 1])
                        else:
                            u = p3c.tile([P, W2N], F32, tag="gelu_u")
                            nc.vector.tensor_add(
                                u[:], ps1[:],
                                f1bs[:, m1:m1 + 1].to_broadcast((P, W2N)))
                            t3 = p3c.tile([P, W2N], F32, tag="gelu_t3")
                            nc.vector.tensor_mul(t3[:], u[:], u[:])
                            nc.vector.tensor_mul(t3[:], t3[:], u[:])
                            nc.vector.scalar_tensor_tensor(
                                t3[:], t3[:], 0.044715, u[:],
                                mybir.AluOpType.mult, mybir.AluOpType.add)
                            nc.scalar.activation(t3[:], t3[:], AF.Tanh,
                                                 scale=0.7978845608028654)
                            nc.vector.scalar_tensor_tensor(
                                t3[:], t3[:], 1.0, u[:],
                                mybir.AluOpType.add, mybir.AluOpType.mult)
                            nc.vector.tensor_scalar_mul(h2c, t3[:], 0.5)
                    for pm in range(KC):
                        ps2 = pp3q.tile([P, W2N], F32, tag="ps2")
                        for m1 in range(M1T):
                            nc.tensor.matmul(
                                ps2[:], w2sb[:, m1, P * pm:P * (pm + 1)],
                                h2a[:, m1, :], start=(m1 == 0), stop=False)
                        nc.tensor.matmul(
                            ps2[:], bias2[:1, C + P * pm:C + P * (pm + 1)],
                            ones_b[:1, :W2N], start=False, stop=True)
                        xps = p3c.tile([P, 2, N], F32, tag="xps")
                        for wh in range(2):
                            nc.sync.dma_start(xps[:, wh, :],
                                              xpd[w0 + wh, :, pm, :])
                        ot = p3c.tile([P, 2, N], F32, tag="ot")
                        nc.vector.tensor_add(
                            ot[:], xps[:],
                            ps2[:].rearrange("p (u n) -> p u n", n=N))
                        for wh in range(2):
                            nc.sync.dma_start(outT[w0 + wh, :, pm, :],
                                              ot[:, wh, :])

    nc.compile()
    return nc


# ---------------------------------------------------------------------------
# host side
# ---------------------------------------------------------------------------

def _qk_colmap():
    m = np.full(2 * H * HS, -1, np.int64)
    for h in range(H):
        m[HS * h:HS * h + DH] = np.arange(72 * h, 72 * h + 72)
        m[H * HS + HS * h:H * HS + HS * h + DH] = \
            np.arange(C + 72 * h, C + 72 * h + 72)
    return m


def _prep_core_inputs(x_c, c_c, wdict):
    """x_c: [nw, N, C], c_c: [nw, C] -> per-core input map"""
    nw = x_c.shape[0]
    xT = np.ascontiguousarray(
        x_c.transpose(0, 2, 1).reshape(nw, KC, P, N).transpose(
            0, 2, 1, 3)).astype(np.float32)
    caug = np.zeros((nw, 1280), np.float32)
    caug[:, :C] = c_c
    caug[:, C] = 1.0
    cT = np.ascontiguousarray(caug.T.reshape(10, P, nw)).astype(NPBF16)
    return {"xT": xT, "cT": cT, **wdict}


def _prep_weights(qkv_w, qkv_b, qkvt_w, qkvt_b, rpb_table, rel_idx,
                  proj_w, proj_b, fc1_w, fc1_b, fc2_w, fc2_b):
    qkmap = _qk_colmap()
    amap = np.concatenate([qkmap, np.arange(2 * C, 3 * C)])  # 4224 cols
    valid = amap >= 0

    wct = np.zeros((1280, 4224), np.float32)
    wct[:C, valid] = qkvt_w[amap[valid], :].T
    wct[C, valid] = (qkv_b + qkvt_b)[amap[valid]]
    wct = wct.reshape(10, P, 4224).astype(NPBF16)

    nqk = 2 * H * HS
    wqkT = np.zeros((C, nqk), np.float32)
    wqkT[:, valid[:nqk]] = qkv_w[qkmap[valid[:nqk]], :].T
    wqk = np.ascontiguousarray(
        wqkT.reshape(KC, P, QKM, P).transpose(2, 1, 0, 3)).astype(NPBF16)

    wv = np.ascontiguousarray(
        qkv_w[2 * C:, :].T.reshape(KC, P, 4, 288).transpose(
            2, 1, 0, 3)).astype(NPBF16)

    bias = rpb_table[rel_idx]                      # [N(n), N(m), H]
    expb = np.ascontiguousarray(
        np.exp(bias).transpose(2, 1, 0).reshape(H, 2, P, N).transpose(
            0, 2, 1, 3)).astype(NPBF16)

    wp_sl = np.zeros((P, H, C), np.float32)        # [slot-row d, head, p]
    for h in range(H):
        wp_sl[1:73, h, :] = proj_w[:, 72 * h:72 * h + 72].T
    wps = np.ascontiguousarray(
        wp_sl.reshape(P, H, KC, P).transpose(2, 0, 1, 3)).astype(NPBF16)

    w1c = np.ascontiguousarray(
        fc1_w.T.reshape(KC, P, M1T, P).transpose(2, 1, 0, 3)).astype(NPBF16)
    w2 = np.ascontiguousarray(
        fc2_w.T.reshape(M1T, P, C)).astype(NPBF16)
    f1b = np.ascontiguousarray(fc1_b.reshape(M1T, P).T).astype(np.float32)
    b2 = np.concatenate([proj_b, fc2_b]).reshape(1, 2 * C).astype(NPBF16)

    return {"wct": wct, "wqk": wqk, "wv": wv, "expb": expb, "wps": wps,
            "w1c": w1c, "w2": w2, "f1b": f1b, "b2": b2}


_PROGRAM = None


def kernel(x, c, qkv_w, qkv_b, qkvt_w, qkvt_b, rpb_table, proj_w, proj_b,
           fc1_w, fc1_b, fc2_w, fc2_b, rel_idx, _trace=False):
    global _PROGRAM
    x = np.asarray(x, np.float32)
    c = np.asarray(c, np.float32)
    wdict = _prep_weights(
        np.asarray(qkv_w, np.float32), np.asarray(qkv_b, np.float32),
        np.asarray(qkvt_w, np.float32), np.asarray(qkvt_b, np.float32),
        np.asarray(rpb_table, np.float32), np.asarray(rel_idx),
        np.asarray(proj_w, np.float32), np.asarray(proj_b, np.float32),
        np.asarray(fc1_w, np.float32), np.asarray(fc1_b, np.float32),
        np.asarray(fc2_w, np.float32), np.asarray(fc2_b, np.float32))

    if _PROGRAM is None:
        _PROGRAM = build_program(NW)
    nc = _PROGRAM

    in_maps = []
    for core in range(NCORES):
        sl = slice(core * NW, (core + 1) * NW)
        in_maps.append(_prep_core_inputs(x[sl], c[sl], wdict))

    res = bass_utils.run_bass_kernel_spmd(
        nc, in_maps, core_ids=list(range(NCORES)), trace=_trace)

    out = np.empty((B, N, C), np.float32)
    for core in range(NCORES):
        oT = res.results[core]["outT"]            # [NW, P, KC, N]
        out[core * NW:(core + 1) * NW] = \
            oT.transpose(0, 2, 1, 3).reshape(NW, C, N).transpose(0, 2, 1)
    if _trace:
        return out, res
    return out


# revision 24
# speedup vs baseline: 1.1718x; 1.0811x over previous
"""DiffiT transformer block kernel for 8 Trainium2 NeuronCores.

Data-parallel over the B=64 window axis (8 windows per core). Activations
are feature-major ([channel, token]) so every linear contracts over the
SBUF partition axis. Q/K stay feature-major with heads packed at a 96-row
stride (so each head's 72 rows sit at 32-aligned partition bases and the
per-head score matmuls can slice them legally); V is produced token-major
into per-head slots with an appended ones-column, so O^T = V_aug.T @ P^T
yields the softmax denominator as row 72. Per-token scalars (LN mean/rstd,
softmax 1/l) are broadcast across partitions with K=1 ones-matmuls on the
PE. Dense matmuls run bf16; the residual stream stays fp32; small fixup
matmuls use float32r (full-rate fp32 at free-dim >= 256).

All biases and the time-token conditioning (c @ qkvt^T + biases) enter as
rank-1 (K=1) matmul fixups folded into the PSUM accumulations.
"""

import math
from contextlib import ExitStack

import numpy as np
import ml_dtypes

import concourse.bass as bass
import concourse.mybir as mybir
import concourse.tile as tile
from concourse import bacc
from concourse import bass_utils

F32 = mybir.dt.float32
F32R = mybir.dt.float32r
BF16 = mybir.dt.bfloat16
NPBF16 = ml_dtypes.bfloat16
AF = mybir.ActivationFunctionType

P = 128
WS = 16
N = 256            # tokens per window
C = 1152           # hidden
H = 16             # heads
DH = 72            # head dim
HS = 96            # head stride in the QK packing (32-aligned, >= DH)
MLP = 4608
EPS = 1e-6
B = 64
NCORES = 8
NW = B // NCORES   # windows per core
KC = C // P        # 9  k-tiles over the hidden dim
QKM = 2 * H * HS // P   # 24 m-tiles over packed Q+K (96-stride)
KOFF = QKM // 2    # first K-side m-tile
M1T = MLP // P     # 36 fc1 row tiles
SCALE = 1.0 / math.sqrt(DH)


def _r(ap):
    """view a 4-byte fp32 AP as float32r for full-rate PE matmuls"""
    return ap.bitcast(F32R)


def _qk_pieces(h):
    """32-aligned partition pieces covering head h's 72 rows in the
    96-stride packing: [(subtile, base, length), ...]; piece legality:
    base 0 any len, base 64 len<=64, base 32/96 len<=32."""
    start, end = HS * h, HS * h + DH
    out = []
    while start < end:
        sub, base = divmod(start, P)
        ln = min(end - start, P - base)
        if base == 64:
            ln = min(ln, 64)
        elif base in (32, 96):
            ln = min(ln, 32)
        elif base != 0:
            raise AssertionError(base)
        out.append((sub, base, ln))
        start += ln
    return out


def build_program(nw=NW, sim_gelu=False):
    nc = bacc.Bacc("TRN2", target_bir_lowering=False, debug=False,
                   num_devices=NCORES)

    # register the layernorm epsilon as a const AP (activation float biases
    # other than 0.0/1.0 need one), same pattern as Bass.__init__
    eps_t = nc.alloc_sbuf_tensor("const-eps", [P, 1], F32)
    nc.gpsimd.memset(eps_t.ap(), EPS)
    nc.const_aps.aps[(F32, EPS)] = eps_t.ap()
    nc.all_engine_barrier()

    def din(name, shape, dt):
        return nc.dram_tensor(name, shape, dt, kind="ExternalInput").ap()

    xT = din("xT", [nw, P, KC, N], F32)          # x, feature-major
    cT = din("cT", [10, P, nw], BF16)            # c augmented with ones row
    wct = din("wct", [10, P, 4224], BF16)        # qkvt^T reordered + bias row
    wqk = din("wqk", [QKM, P, KC, P], BF16)      # qkv^T QK part, 96-stride
    wv = din("wv", [4, P, KC, 288], BF16)        # qkv^T V part, chunk-major
    expb = din("expb", [H, P, 2, N], BF16)       # exp(rel-pos bias)^T per head
    wps = din("wps", [KC, P, H, P], BF16)        # proj^T, head-slot padded
    w1c = din("w1c", [M1T, P, KC, P], BF16)      # fc1^T pre-chunked
    w2 = din("w2", [M1T, P, C], BF16)            # fc2^T
    f1b = din("f1b", [P, M1T], F32)              # fc1 bias, per-partition
    b2 = din("b2", [1, 2 * C], BF16)             # proj_b ++ fc2_b
    outT = nc.dram_tensor("outT", [nw, P, KC, N], F32,
                          kind="ExternalOutput").ap()

    NPAIR = nw // 2
    W2N = 2 * N        # tokens per window pair

    with tile.TileContext(nc) as tc, ExitStack() as ctx:
        keep = ctx.enter_context(tc.tile_pool(name="keep", bufs=1))
        dram = ctx.enter_context(tc.tile_pool(name="dram", bufs=1,
                                              space="DRAM"))

        ones_b = keep.tile([1, W2N], BF16, tag="ones_b")  # bf16 rhs of K=1
        ones_c = keep.tile([P, 1], BF16, tag="ones_c")    # lhsT of column sums
        nc.gpsimd.memset(ones_b[:], 1.0)
        nc.gpsimd.memset(ones_c[:], 1.0)
        bias2 = keep.tile([1, 2 * C], BF16, tag="bias2")
        nc.sync.dma_start(bias2[:], b2[:])
        f1bs = keep.tile([P, M1T], F32, tag="f1bs")
        nc.sync.dma_start(f1bs[:], f1b[:])

        tdram = dram.tile([nw, 4224], BF16)
        xpd = dram.tile([nw, P, KC, N], F32)     # x after attention branch

        # ---- phase 0: conditioning T = c_aug @ W_ct ----------------------
        with tc.tile_pool(name="ph0", bufs=2) as p0, \
             tc.tile_pool(name="ph0p", bufs=2, space="PSUM") as pp0:
            caug = p0.tile([P, 10, nw], BF16, tag="caug")
            nc.sync.dma_start(caug[:], cT.rearrange("k p w -> p k w"))
            tsb = p0.tile([8, 4224], BF16, tag="tsb")
            for i in range(9):
                n0, nl = i * 512, min(512, 4224 - i * 512)
                tps = pp0.tile([8, 512], F32, tag="tps")
                for k in range(10):
                    wt = p0.tile([P, 512], BF16, tag="wctt")
                    nc.sync.dma_start(wt[:, :nl], wct[k, :, n0:n0 + nl])
                    nc.tensor.matmul(tps[:nw, :nl], caug[:, k, :], wt[:, :nl],
                                     start=(k == 0), stop=(k == 9))
                nc.scalar.activation(tsb[:nw, n0:n0 + nl], tps[:nw, :nl],
                                     AF.Copy)
            nc.sync.dma_start(tdram[:, :], tsb[:nw, :])

        # ---- layernorm for a window pair -> PSUM broadcast [P, W2N] ------
        # acc-tile layout: [:, :N]+[:, N:] hold the two windows; returns one
        # [P, W2N] psum tile pair (rstd bcast, -mean*rstd bcast)
        def ln_pair(pool, accp, xw, tag):
            xb = pool.tile([P, W2N], BF16, tag=tag + "xb")
            xsq = pool.tile([P, W2N], BF16, tag=tag + "xsq")
            ms0 = accp.tile([1, W2N], F32, tag="acc")
            ms1 = accp.tile([1, W2N], F32, tag="acc")
            for s in range(KC):
                nc.gpsimd.tensor_copy(xb[:], xw[:, s, :])
                nc.vector.tensor_mul(xsq[:], xw[:, s, :], xw[:, s, :])
                nc.tensor.matmul(ms0[:], ones_c[:], xb[:],
                                 start=(s == 0), stop=(s == KC - 1))
                nc.tensor.matmul(ms1[:], ones_c[:], xsq[:],
                                 start=(s == 0), stop=(s == KC - 1))
            mean = pool.tile([1, W2N], F32, tag=tag + "mean")
            e2 = pool.tile([1, W2N], F32, tag=tag + "e2")
            nc.vector.tensor_scalar_mul(mean[:], ms0[:], 1.0 / C)
            nc.vector.tensor_scalar_mul(e2[:], ms1[:], 1.0 / C)
            var = pool.tile([1, W2N], F32, tag=tag + "var")
            nc.vector.tensor_mul(var[:], mean[:], mean[:])
            nc.vector.tensor_sub(var[:], e2[:], var[:])
            sd = pool.tile([1, W2N], F32, tag=tag + "sd")
            nc.scalar.activation(sd[:], var[:], AF.Sqrt, bias=EPS)
            rsf = pool.tile([1, W2N], F32, tag=tag + "rsf")
            nc.vector.reciprocal_approx_fast(rsf[:], sd[:])
            rstd = pool.tile([1, W2N], BF16, tag=tag + "rstd")
            nc.gpsimd.tensor_copy(rstd[:], rsf[:])
            bneg = pool.tile([1, W2N], BF16, tag=tag + "bneg")
            nc.vector.scalar_tensor_tensor(
                bneg[:], mean[:], -1.0, rstd[:],
                mybir.AluOpType.mult, mybir.AluOpType.mult)
            bc = accp.tile([P, W2N], F32, tag="acc")
            nc.tensor.matmul(bc[:], ones_b[:1, :P], rstd[:],
                             start=True, stop=True)
            bb = accp.tile([P, W2N], F32, tag="acc")
            nc.tensor.matmul(bb[:], ones_b[:1, :P], bneg[:],
                             start=True, stop=True)
            return bc, bb

        # ==== attention superphase: per pair LN1 -> QKV -> attn -> proj ===
        with tc.tile_pool(name="sp", bufs=2) as sp, \
             tc.tile_pool(name="sp1", bufs=1) as sp1, \
             tc.tile_pool(name="spw", bufs=2) as spw, \
             tc.tile_pool(name="sps", bufs=3) as sps, \
             tc.tile_pool(name="spr", bufs=2) as spr, \
             tc.tile_pool(name="accp", bufs=8, space="PSUM") as accp:
            for pr in range(NPAIR):
                w0 = 2 * pr
                xw = sp1.tile([P, KC, W2N], F32, tag="xw")
                for wh in range(2):
                    nc.sync.dma_start(
                        xw[:, :, wh * N:(wh + 1) * N], xT[w0 + wh])
                t1q = sp1.tile([1, 2, 3072], BF16, tag="t1q")
                nc.sync.dma_start(t1q[:], tdram[w0:w0 + 2, :3072]
                                  .unsqueeze(0))
                t1v = sp1.tile([1, 2, 1152], BF16, tag="t1v")
                nc.sync.dma_start(t1v[:], tdram[w0:w0 + 2, 3072:]
                                  .unsqueeze(0))
                bc, bb = ln_pair(spw, accp, xw, "ln1")
                hw = sp.tile([P, KC, W2N], BF16, tag="hw")
                for s in range(KC):
                    nc.vector.tensor_mul(hw[:, s, :], xw[:, s, :], bc[:])
                    nc.vector.tensor_add(hw[:, s, :], hw[:, s, :], bb[:])
                # QK (96-stride packed), N = both windows
                qkst = sp.tile([P, QKM, W2N], BF16, tag="qkst")
                for m in range(QKM):
                    wt = spw.tile([P, KC, P], BF16, tag="wqkt")
                    nc.sync.dma_start(wt[:], wqk[m])
                    qs = accp.tile([P, W2N], F32, tag="acc")
                    for k in range(KC):
                        nc.tensor.matmul(qs[:], wt[:, k, :], hw[:, k, :],
                                         start=(k == 0), stop=False)
                    msl = slice(P * m, P * (m + 1))
                    nc.tensor.matmul(qs[:, :N], t1q[:1, 0, msl],
                                     ones_b[:1, :N], start=False, stop=False)
                    nc.tensor.matmul(qs[:, N:], t1q[:1, 1, msl],
                                     ones_b[:1, :N], start=False, stop=True)
                    nc.scalar.activation(qkst[:, m, :], qs[:], AF.Copy)
                # V token-major into per-head slots (ones in col 0)
                vsl = sp.tile([P, 2, 2, H, 73], BF16, tag="vsl")
                nc.vector.memset(vsl[:, :, :, :, 0:1], 1.0)
                for nch in range(4):
                    wvt = spw.tile([P, KC, 288], BF16, tag="wvt")
                    nc.sync.dma_start(wvt[:], wv[nch])
                    for tch in range(4):       # token chunks of the pair
                        wh, ms = divmod(tch, 2)
                        vs = accp.tile([P, W2N], F32, tag="acc")
                        tsl = slice(tch * P, (tch + 1) * P)
                        for k in range(KC):
                            nc.tensor.matmul(vs[:, :288], hw[:, k, tsl],
                                             wvt[:, k, :],
                                             start=(k == 0), stop=False)
                        nc.tensor.matmul(
                            vs[:, :288], ones_b[:1, :P],
                            t1v[:1, wh, 288 * nch:288 * (nch + 1)],
                            start=False, stop=True)
                        nc.scalar.activation(
                            vsl[:, wh, ms, 4 * nch:4 * nch + 4, 1:73],
                            vs[:, :288].rearrange("p (h d) -> p h d", d=72),
                            AF.Copy)
                # attention per (window, head)
                ost = sp1.tile([P, H, W2N], BF16, tag="ost")
                nc.gpsimd.memset(ost[64:, :, :], 0.0)
                for wh in range(2):
                    nsl = slice(wh * N, (wh + 1) * N)
                    for h in range(H):
                        ebt = sps.tile([P, 2, N], BF16, tag="ebt")
                        nc.sync.dma_start(ebt[:], expb[h])
                        pt = sps.tile([P, 2, N], BF16, tag="pt")
                        pieces = _qk_pieces(h)
                        po = accp.tile([P, W2N], F32, tag="acc")
                        for ms in range(2):
                            ssp = accp.tile([P, W2N], F32, tag="acc")
                            msl = slice(wh * N + ms * P, wh * N + (ms + 1) * P)
                            for i, (sub, base, ln) in enumerate(pieces):
                                nc.tensor.matmul(
                                    ssp[:, :N],
                                    qkst[base:base + ln, KOFF + sub, msl],
                                    qkst[base:base + ln, sub, nsl],
                                    start=(i == 0),
                                    stop=(i == len(pieces) - 1),
                                    tile_position=(base, 0))
                            nc.scalar.activation(pt[:, ms, :], ssp[:, :N],
                                                 AF.Exp, scale=SCALE)
                            nc.vector.tensor_mul(pt[:, ms, :], pt[:, ms, :],
                                                 ebt[:, ms, :])
                        for ms in range(2):
                            nc.tensor.matmul(po[:73, :N],
                                             vsl[:, wh, ms, h, :],
                                             pt[:, ms, :],
                                             start=(ms == 0), stop=(ms == 1))
                        linv = spr.tile([1, N], F32, tag="linv")
                        nc.vector.reciprocal_approx_fast(linv[:], po[0:1, :N])
                        pbs = spr.tile([P, N], F32, tag="pbs")
                        nc.gpsimd.partition_broadcast(pbs[:73, :], linv[:],
                                                      channels=73)
                        nc.scalar.activation(ost[:73, h, nsl], po[:73, :N],
                                             AF.Copy)
                        nc.vector.tensor_mul(ost[:73, h, nsl],
                                             ost[:73, h, nsl], pbs[:73, :])
                # proj + residual, both windows at once
                for pc in range(KC):
                    wpt = spw.tile([P, H, P], BF16, tag="wpt")
                    nc.sync.dma_start(wpt[:], wps[pc])
                    yps = accp.tile([P, W2N], F32, tag="acc")
                    for h in range(H):
                        nc.tensor.matmul(yps[:], wpt[:, h, :], ost[:, h, :],
                                         start=(h == 0), stop=False)
                    nc.tensor.matmul(yps[:], bias2[:1, P * pc:P * (pc + 1)],
                                     ones_b[:1, :W2N], start=False, stop=True)
                    xres = spw.tile([P, 2, N], F32, tag="xres")
                    for wh in range(2):
                        nc.sync.dma_start(xres[:, wh, :],
                                          xT[w0 + wh, :, pc, :])
                    nc.vector.tensor_add(
                        xres[:], xres[:],
                        yps[:].rearrange("p (u n) -> p u n", n=N))
                    for wh in range(2):
                        nc.sync.dma_start(xpd[w0 + wh, :, pc, :],
                                          xres[:, wh, :])

        # ---- phase 3a: LN2 -> h' (bf16) for all windows ------------------
        with tc.tile_pool(name="hp", bufs=1) as hppool:
            hpall = hppool.tile([P, KC, nw, N], BF16, tag="hpall")
            with tc.tile_pool(name="ph3a", bufs=2) as p3a, \
                 tc.tile_pool(name="ap3", bufs=4, space="PSUM") as ap3:
                for pr in range(NPAIR):
                    w0 = 2 * pr
                    xpw = p3a.tile([P, KC, W2N], F32, tag="xpw")
                    for wh in range(2):
                        nc.sync.dma_start(
                            xpw[:, :, wh * N:(wh + 1) * N], xpd[w0 + wh])
                    bc, bb = ln_pair(p3a, ap3, xpw, "ln2")
                    xpv = xpw[:].rearrange("p s (u n) -> p s u n", n=N)
                    for s in range(KC):
                        nc.vector.tensor_mul(
                            hpall[:, s, w0:w0 + 2, :], xpv[:, s, :, :],
                            bc[:].rearrange("p (u n) -> p u n", n=N))
                        nc.vector.tensor_add(
                            hpall[:, s, w0:w0 + 2, :],
                            hpall[:, s, w0:w0 + 2, :],
                            bb[:].rearrange("p (u n) -> p u n", n=N))

            # ---- phase 3b: fc1 -> gelu -> fc2 -> residual, per pair ------
            with tc.tile_pool(name="ph3b", bufs=1) as p3b, \
                 tc.tile_pool(name="ph3w", bufs=2) as p3w, \
                 tc.tile_pool(name="ph3c", bufs=3) as p3c, \
                 tc.tile_pool(name="ph3bp", bufs=2, space="PSUM") as pp3b, \
                 tc.tile_pool(name="ph3bq", bufs=2, space="PSUM") as pp3q:
                w2sb = p3b.tile([P, M1T, C], BF16, tag="w2sb")
                nc.sync.dma_start(w2sb[:], w2.rearrange("k p n -> p k n"))
                for pr in range(NPAIR):
                    w0 = 2 * pr
                    h2a = p3b.tile([P, M1T, W2N], BF16, tag="h2a")
                    hin = hpall[:, :, w0:w0 + 2, :].rearrange(
                        "p s u n -> p s (u n)")
                    for m1 in range(M1T):
                        w1t = p3w.tile([P, KC, P], BF16, tag="w1t")
                        nc.sync.dma_start(w1t[:], w1c[m1])
                        ps1 = pp3b.tile([P, W2N], F32, tag="ps1")
                        for k in range(KC):
                            nc.tensor.matmul(ps1[:], w1t[:, k, :],
                                             hin[:, k, :],
                                             start=(k == 0),
                                             stop=(k == KC - 1))
                        h2c = h2a[:, m1, :]
                        if not sim_gelu:
                            nc.scalar.activation(h2c, ps1[:],
                                                 AF.Gelu_apprx_tanh,
                                                 bias=f1bs[:, m1:m1 + 1])
                        else:
                            u = p3c.tile([P, W2N], F32, tag="gelu_u")
                            nc.vector.tensor_add(
                                u[:], ps1[:],
                                f1bs[:, m1:m1 + 1].to_broadcast((P, W2N)))
                            t3 = p3c.tile([P, W2N], F32, tag="gelu_t3")
                            nc.vector.tensor_mul(t3[:], u[:], u[:])
                            nc.vector.tensor_mul(t3[:], t3[:], u[:])
                            nc.vector.scalar_tensor_tensor(
                                t3[:], t3[:], 0.044715, u[:],
                                mybir.AluOpType.mult, mybir.AluOpType.add)
                            nc.scalar.activation(t3[:], t3[:], AF.Tanh,
                                                 scale=0.7978845608028654)
                            nc.vector.scalar_tensor_tensor(
                                t3[:], t3[:], 1.0, u[:],
                                mybir.AluOpType.add, mybir.AluOpType.mult)
                            nc.vector.tensor_scalar_mul(h2c, t3[:], 0.5)
                    for pm in range(KC):
                        ps2 = pp3q.tile([P, W2N], F32, tag="ps2")
                        for m1 in range(M1T):
                            nc.tensor.matmul(
                                ps2[:], w2sb[:, m1, P * pm:P * (pm + 1)],
                                h2a[:, m1, :], start=(m1 == 0), stop=False)
                        nc.tensor.matmul(
                            ps2[:], bias2[:1, C + P * pm:C + P * (pm + 1)],
                            ones_b[:1, :W2N], start=False, stop=True)
                        xps = p3c.tile([P, 2, N], F32, tag="xps")
                        for wh in range(2):
                            nc.sync.dma_start(xps[:, wh, :],
                                              xpd[w0 + wh, :, pm, :])
                        ot = p3c.tile([P, 2, N], F32, tag="ot")
                        nc.vector.tensor_add(
                            ot[:], xps[:],
                            ps2[:].rearrange("p (u n) -> p u n", n=N))
                        for wh in range(2):
                            nc.sync.dma_start(outT[w0 + wh, :, pm, :],
                                              ot[:, wh, :])

    nc.compile()
    return nc


# ---------------------------------------------------------------------------
# host side
# ---------------------------------------------------------------------------

def _qk_colmap():
    m = np.full(2 * H * HS, -1, np.int64)
    for h in range(H):
        m[HS * h:HS * h + DH] = np.arange(72 * h, 72 * h + 72)
        m[H * HS + HS * h:H * HS + HS * h + DH] = \
            np.arange(C + 72 * h, C + 72 * h + 72)
    return m


def _prep_core_inputs(x_c, c_c, wdict):
    """x_c: [nw, N, C], c_c: [nw, C] -> per-core input map"""
    nw = x_c.shape[0]
    xT = np.ascontiguousarray(
        x_c.transpose(0, 2, 1).reshape(nw, KC, P, N).transpose(
            0, 2, 1, 3)).astype(np.float32)
    caug = np.zeros((nw, 1280), np.float32)
    caug[:, :C] = c_c
    caug[:, C] = 1.0
    cT = np.ascontiguousarray(caug.T.reshape(10, P, nw)).astype(NPBF16)
    return {"xT": xT, "cT": cT, **wdict}


def _prep_weights(qkv_w, qkv_b, qkvt_w, qkvt_b, rpb_table, rel_idx,
                  proj_w, proj_b, fc1_w, fc1_b, fc2_w, fc2_b):
    qkmap = _qk_colmap()
    amap = np.concatenate([qkmap, np.arange(2 * C, 3 * C)])  # 4224 cols
    valid = amap >= 0

    wct = np.zeros((1280, 4224), np.float32)
    wct[:C, valid] = qkvt_w[amap[valid], :].T
    wct[C, valid] = (qkv_b + qkvt_b)[amap[valid]]
    wct = wct.reshape(10, P, 4224).astype(NPBF16)

    nqk = 2 * H * HS
    wqkT = np.zeros((C, nqk), np.float32)
    wqkT[:, valid[:nqk]] = qkv_w[qkmap[valid[:nqk]], :].T
    wqk = np.ascontiguousarray(
        wqkT.reshape(KC, P, QKM, P).transpose(2, 1, 0, 3)).astype(NPBF16)

    wv = np.ascontiguousarray(
        qkv_w[2 * C:, :].T.reshape(KC, P, 4, 288).transpose(
            2, 1, 0, 3)).astype(NPBF16)

    bias = rpb_table[rel_idx]                      # [N(n), N(m), H]
    expb = np.ascontiguousarray(
        np.exp(bias).transpose(2, 1, 0).reshape(H, 2, P, N).transpose(
            0, 2, 1, 3)).astype(NPBF16)

    wp_sl = np.zeros((P, H, C), np.float32)        # [slot-row d, head, p]
    for h in range(H):
        wp_sl[1:73, h, :] = proj_w[:, 72 * h:72 * h + 72].T
    wps = np.ascontiguousarray(
        wp_sl.reshape(P, H, KC, P).transpose(2, 0, 1, 3)).astype(NPBF16)

    w1c = np.ascontiguousarray(
        fc1_w.T.reshape(KC, P, M1T, P).transpose(2, 1, 0, 3)).astype(NPBF16)
    w2 = np.ascontiguousarray(
        fc2_w.T.reshape(M1T, P, C)).astype(NPBF16)
    f1b = np.ascontiguousarray(fc1_b.reshape(M1T, P).T).astype(np.float32)
    b2 = np.concatenate([proj_b, fc2_b]).reshape(1, 2 * C).astype(NPBF16)

    return {"wct": wct, "wqk": wqk, "wv": wv, "expb": expb, "wps": wps,
            "w1c": w1c, "w2": w2, "f1b": f1b, "b2": b2}


_PROGRAM = None


def kernel(x, c, qkv_w, qkv_b, qkvt_w, qkvt_b, rpb_table, proj_w, proj_b,
           fc1_w, fc1_b, fc2_w, fc2_b, rel_idx, _trace=False):
    global _PROGRAM
    x = np.asarray(x, np.float32)
    c = np.asarray(c, np.float32)
    wdict = _prep_weights(
        np.asarray(qkv_w, np.float32), np.asarray(qkv_b, np.float32),
        np.asarray(qkvt_w, np.float32), np.asarray(qkvt_b, np.float32),
        np.asarray(rpb_table, np.float32), np.asarray(rel_idx),
        np.asarray(proj_w, np.float32), np.asarray(proj_b, np.float32),
        np.asarray(fc1_w, np.float32), np.asarray(fc1_b, np.float32),
        np.asarray(fc2_w, np.float32), np.asarray(fc2_b, np.float32))

    if _PROGRAM is None:
        _PROGRAM = build_program(NW)
    nc = _PROGRAM

    in_maps = []
    for core in range(NCORES):
        sl = slice(core * NW, (core + 1) * NW)
        in_maps.append(_prep_core_inputs(x[sl], c[sl], wdict))

    res = bass_utils.run_bass_kernel_spmd(
        nc, in_maps, core_ids=list(range(NCORES)), trace=_trace)

    out = np.empty((B, N, C), np.float32)
    for core in range(NCORES):
        oT = res.results[core]["outT"]            # [NW, P, KC, N]
        out[core * NW:(core + 1) * NW] = \
            oT.transpose(0, 2, 1, 3).reshape(NW, C, N).transpose(0, 2, 1)
    if _trace:
        return out, res
    return out
